# revision 23
# baseline (speedup 1.0000x reference)
"""LocationAwareAttention TRN2 kernel — 8-core SPMD, head+batch sharded.

Sharding: core c handles batch b = c//4 and heads 4*(c%4) .. 4*(c%4)+3.
Each core computes qkv for its 4 heads, gated-softmax attention, and a
partial output projection (its heads' slice of the contraction). The host
sums the 4 partials per batch and adds proj_b.

On-chip layout (per core):
  qkvT  [768, 2048]  rows = [q_h0..q_h3 | k_h0..k_h3 | v_h0..v_h3] x 64 dims
  S^T   [keys, queries] chunks, gate applied as per-partition (key) scale
        inside the Exp activation; softmax denominator via an appended
        ones-column on v (row 64 of the out^T accumulation).
  out^T [65, q] per head accumulated over key blocks; row 64 = denom;
        normalized via DVE reciprocal + a K=1 PE matmul that broadcasts the
        reciprocal row across partitions + DVE mul.
"""

import ml_dtypes
import numpy as np

import concourse.bass as bass
import concourse.mybir as mybir
import concourse.tile as tile
from concourse.bass_utils import run_bass_kernel_spmd
from concourse.vector_clock import ScopedClock

B, N, C = 2, 2048, 1024
H, HD = 16, 64
GH = C // 4
P = 128
HPC = 4          # heads per core
N_CORES = 8
NKB = N // P     # 16 key blocks
NQC = N // P     # 16 query 128-chunks
SCALE = HD ** -0.5

f32 = mybir.dt.float32
f32r = mybir.dt.float32r
bf16 = mybir.dt.bfloat16
AF = mybir.ActivationFunctionType


class SplitDrainTileContext(tile.TileContext):
    """Workaround: this container's walrus rejects >1 sync wait on the Tile
    exit InstDrain ("Too many sync wait commands"). Split the final drain's
    waits across chained single-wait drains."""

    def _drain_and_barrier(self, tick_clock, wait_clock):
        nc = self.nc
        drain_inst = nc.sync.drain()
        wait_clock.add_sem_waits(
            drain_inst.ins, ScopedClock({None: tick_clock.global_clock})
        )
        si = drain_inst.ins.sync_info
        waits = list(si.on_wait) if si and si.on_wait else []
        if len(waits) > 1:
            SyncInfo = type(si)
            drain_inst.ins.sync_info = SyncInfo(
                on_wait=waits[:1], on_update=list(si.on_update)
            )
            for i in range(1, len(waits)):
                extra = nc.sync.drain()
                esi = extra.ins.sync_info
                upd = list(esi.on_update) if esi and esi.on_update else []
                extra.ins.sync_info = SyncInfo(on_wait=waits[i : i + 1], on_update=upd)

        nc.all_engine_barrier()
        assert self.sems is not None
        popped = nc._tile_sem_poison_stack.pop()
        assert popped is self._sem_poison
        nc.clear_and_free_semaphores(list(self.sems.allocated().values()))
        nc.all_engine_barrier()


def split_excess_waits(nc, max_waits=1):
    """This container's walrus rejects instructions carrying more than one
    sync wait. Move excess waits onto same-engine InstNoOps inserted just
    before the instruction (engine streams are in-order, so waiting earlier
    on the same engine is equivalent)."""
    n = 0
    for bb in nc.main_func.blocks:
        il = bb.instructions
        out = []
        changed = False
        for ins in il:
            si = ins.sync_info
            waits = list(si.on_wait) if si and si.on_wait else []
            if len(waits) > max_waits:
                SyncInfo = type(si)
                for i in range(0, len(waits) - max_waits):
                    nop = mybir.InstNoOp(
                        name=f"I-wsplit-{n}",
                        engine=ins.engine,
                        bass_nofuse=True,
                        sync_info=SyncInfo(on_wait=[waits[i]], on_update=[]),
                    )
                    n += 1
                    nc.register_instruction(nop)
                    out.append(nop)
                ins.sync_info = SyncInfo(
                    on_wait=waits[len(waits) - max_waits :],
                    on_update=list(si.on_update),
                )
                changed = True
            out.append(ins)
        if changed:
            bb.instructions = out
    return nc


def build_nc(reps: int = 1):
    nc = bass.Bass()

    xbT = nc.dram_tensor("xbT", [C, N], bf16, kind="ExternalInput")
    wqkvT = nc.dram_tensor("wqkvT", [C, 6 * P], bf16, kind="ExternalInput")
    projT = nc.dram_tensor("projT", [2 * P, C], bf16, kind="ExternalInput")
    # all small f32 constants packed into one tensor -> one startup DMA:
    # cols [qkvb(6) | gmask(16) | g1w(256) | g1b(256) | g2w(256) | g2b(1)]
    GC = 6 + NKB + 3 * GH + 1
    gconst = nc.dram_tensor("gconst", [P, GC], f32, kind="ExternalInput")
    ident_in = nc.dram_tensor("ident_in", [P, P], bf16, kind="ExternalInput")
    ones_in = nc.dram_tensor("ones_in", [1, 64], f32, kind="ExternalInput")
    vones_in = nc.dram_tensor("vones_in", [P, HPC, NKB, 1], bf16, kind="ExternalInput")
    out = nc.dram_tensor("out", [N, C], bf16, kind="ExternalOutput")

    with SplitDrainTileContext(nc) as tc:
        with (
            tc.tile_pool(name="const", bufs=1) as const,
            tc.tile_pool(name="xin", bufs=8) as xin,
            tc.tile_pool(name="qkvt", bufs=1) as qkvt,
            tc.tile_pool(name="vext", bufs=1) as vextp,
            tc.tile_pool(name="exps", bufs=34) as expsp,
            tc.tile_pool(name="outt", bufs=1) as outtp,
            tc.tile_pool(name="small", bufs=4) as small,
            tc.tile_pool(name="ostage", bufs=4) as ostage,
            tc.tile_pool(name="gates", bufs=2) as gatesp,
            tc.tile_pool(name="mm", bufs=2, space="PSUM") as mm,
            tc.tile_pool(name="pso", bufs=3, space="PSUM") as pso,
            tc.tile_pool(name="recb", bufs=1, space="PSUM") as recbp,
        ):
            # ---- constant loads: one packed small-const DMA first so the
            # gate MLP can start while the big weight/x streams stream in ----
            gc_sb = const.tile([P, GC], f32, tag="gc")
            nc.sync.dma_start(gc_sb[:], gconst[:])
            qkvb_sb = [gc_sb[:, m : m + 1] for m in range(6)]
            gm_sb = gc_sb[:, 6 : 6 + NKB]
            o1 = 6 + NKB
            g1w_sb = gc_sb[:, o1 : o1 + GH]
            g1b_sb = gc_sb[:, o1 + GH : o1 + 2 * GH]
            g2w_sb = gc_sb[:, o1 + 2 * GH : o1 + 3 * GH]
            g2b_sb = gc_sb[:, o1 + 3 * GH : o1 + 3 * GH + 1]
            ident = const.tile([P, P], bf16, tag="ident")
            nc.sync.dma_start(ident[:], ident_in[:])
            ones_col = const.tile([1, 64], f32r, tag="ones_col")
            nc.sync.dma_start(ones_col[:], ones_in[:].bitcast(f32r))
            w_sb = []
            for kc in range(C // P):
                t = const.tile([P, 6 * P], bf16, tag=f"w{kc}", name=f"w{kc}")
                nc.sync.dma_start(t[:], wqkvT[kc * P : (kc + 1) * P, :])
                w_sb.append(t)
            pj_sb = []
            for kc in range(2):
                t = const.tile([P, C], bf16, tag=f"pj{kc}", name=f"pj{kc}")
                pj_sb.append(t)

            for rep in range(reps):
                x_sb = []
                for kc in range(C // P):
                    t = xin.tile([P, N], bf16, tag="xt")
                    nc.sync.dma_start(t[:], xbT[kc * P : (kc + 1) * P, :])
                    x_sb.append(t)
                if rep == 0:
                    # projT queued after x: it is not needed until the first
                    # proj (~150us in) and would delay the startup x stream
                    for kc in range(2):
                        nc.sync.dma_start(pj_sb[kc][:], projT[kc * P : (kc + 1) * P, :])

                # ---- spatial gate: gatesc[k] = 0.125 * sigmoid(mlp(mask[k])) ----
                gatesc = const.tile([P, NKB], f32, tag="gatesc")
                for kb in range(NKB):
                    m_col = gc_sb[:, 6 + kb : 7 + kb]
                    t1 = gatesp.tile([P, GH], f32, tag="g_t1")
                    nc.vector.tensor_scalar_mul(t1[:], g1w_sb, m_col)
                    nc.vector.tensor_add(t1[:], t1[:], g1b_sb)
                    nc.scalar.activation(t1[:], t1[:], AF.Relu)
                    nc.vector.tensor_mul(t1[:], t1[:], g2w_sb)
                    gp = gatesp.tile([P, 1], f32, tag="g_gp")
                    nc.vector.reduce_sum(gp[:], t1[:], axis=mybir.AxisListType.X)
                    nc.scalar.activation(gp[:], gp[:], AF.Sigmoid, bias=g2b_sb)
                    nc.scalar.mul(gatesc[:, kb : kb + 1], gp[:], SCALE)

                # ---- qkvT = W_sel @ x_b^T  (+bias) : [768, 2048] ----
                # Emitted in (m, nf, half) units of 8 matmuls so S^T+exp
                # work can interleave between units at ~1.7us granularity.
                qkvT_m = {m: qkvt.tile([P, N], bf16, tag=f"qkvT{m}", name=f"qkvT{m}") for m in range(6)}

                def emit_qkv_unit(m, q):
                    # one self-contained 512-column block: 8 accumulating
                    # matmuls into a 1-bank pso tile + fused bias-add drain
                    ps = pso.tile([P, 512], f32, tag="pso", name="qkv_ps")
                    for kc in range(C // P):
                        nc.tensor.matmul(
                            ps[:],
                            lhsT=w_sb[kc][:, m * P : (m + 1) * P],
                            rhs=x_sb[kc][:, q * 512 : (q + 1) * 512],
                            start=(kc == 0),
                            stop=(kc == C // P - 1),
                        )
                    nc.vector.tensor_scalar_add(
                        qkvT_m[m][:, q * 512 : (q + 1) * 512], ps[:], qkvb_sb[m]
                    )

                def emit_qkv(m):
                    for q in range(4):
                        emit_qkv_unit(m, q)

                # ---- attention: software-pipelined over 8 (qh, hp) groups ----
                # Two groups of S^T + gated exp are prologued between qkv
                # emissions so ACT always has a ~16-tile backlog; in the main
                # loop stage A of group g+2 is emitted interleaved per key
                # block with stage B of group g (out^T accumulation).
                outT_hp = [outtp.tile([P, N], bf16, tag=f"outT{i}", name=f"outT{i}") for i in range(2)]
                groups = [(qh, hp) for qh in range(4) for hp in range(2)]

                def st_exp(gi, kb):
                    qh, hp = groups[gi]
                    qm, km = hp, 2 + hp
                    qs = qh * 512
                    ps = mm.tile([P, 1024], f32, tag="mmt", name="st_ps")
                    for par in range(2):
                        nc.tensor.matmul(
                            ps[:, par * 512 : (par + 1) * 512],
                            lhsT=qkvT_m[km][par * 64 : par * 64 + 64, kb * P : (kb + 1) * P],
                            rhs=qkvT_m[qm][par * 64 : par * 64 + 64, qs : qs + 512],
                            start=True,
                            stop=True,
                        )
                    e = expsp.tile([P, 1024], bf16, tag="exps", name="exps_t")
                    nc.scalar.activation(e[:], ps[:], AF.Exp, scale=gatesc[:, kb : kb + 1])
                    return e

                # q/k for head pair 0 first (DMA-paced), then the remaining
                # qkv units interleaved 2 S^T+exp pairs per 8-matmul unit so
                # PE never idles while ACT digests the exp backlog (the
                # 2-slot mm pool caps ACT lookahead, so st emission would
                # otherwise pace PE at exp speed).
                # boot: qkv for m=0,2 processed kc-major across 8 one-bank
                # psum slots (3 pso + 2x2 mm halves + 1 recb) so PE computes
                # at x-stream pace instead of stalling for the full x load
                boot_ps = {}
                slots = []
                for _ in range(3):
                    bp = pso.tile([P, 512], f32, tag="pso", name="boot_ps")
                    slots.append(bp)
                for _ in range(2):
                    bm = mm.tile([P, 1024], f32, tag="mmt", name="boot_mm")
                    slots.append(bm[:, 0:512])
                    slots.append(bm[:, 512:1024])
                br = recbp.tile([P, 512], f32, tag="recb", name="boot_rec")
                slots.append(br)
                boot_mq = [(m, q) for m in (0, 2) for q in range(4)]
                for si, (m, q) in enumerate(boot_mq):
                    boot_ps[m, q] = slots[si]
                for kc in range(C // P):
                    for m, q in boot_mq:
                        nc.tensor.matmul(
                            boot_ps[m, q][:],
                            lhsT=w_sb[kc][:, m * P : (m + 1) * P],
                            rhs=x_sb[kc][:, q * 512 : (q + 1) * 512],
                            start=(kc == 0),
                            stop=(kc == C // P - 1),
                        )
                for m, q in boot_mq:
                    nc.vector.tensor_scalar_add(
                        qkvT_m[m][:, q * 512 : (q + 1) * 512], boot_ps[m, q][:], qkvb_sb[m]
                    )

                exps_g = {0: [], 1: []}
                units = [(m, q) for m in (1, 3, 4, 5) for q in range(4)]
                sts = [(0, kb) for kb in range(NKB)] + [(1, kb) for kb in range(NKB)]
                for ui, (m, q) in enumerate(units):
                    emit_qkv_unit(m, q)
                    for gi, kb in sts[2 * ui : 2 * ui + 2]:
                        exps_g[gi].append(st_exp(gi, kb))

                # ---- v^T -> v transpose, build vext [keys, 65] per head (bf16) ----
                vext = vextp.tile([P, HPC, NKB, 65], bf16, tag="vext")
                nc.sync.dma_start(vext[:, :, :, 64:65], vones_in[:])
                with nc.allow_low_precision(reason="pure transpose, no accumulation"):
                    for vc in range(2):  # qkvT chunks 4,5 hold [v_h0|v_h1], [v_h2|v_h3]
                        for g in range(2):  # groups of 8 key blocks share one psum tile
                            ps = mm.tile([P, 1024], bf16, tag="mmt", name="tr_ps")
                            for kk in range(8):
                                kb = g * 8 + kk
                                nc.tensor.transpose(
                                    ps[:, kk * P : (kk + 1) * P],
                                    qkvT_m[4 + vc][:, kb * P : (kb + 1) * P],
                                    ident[:],
                                )
                            for kk in range(8):
                                kb = g * 8 + kk
                                for half in range(2):
                                    nc.vector.tensor_copy(
                                        vext[:, 2 * vc + half, kb, 0:64],
                                        ps[:, kk * P + half * 64 : kk * P + half * 64 + 64],
                                    )

                for gi in range(len(groups)):
                    qh, hp = groups[gi]
                    qs = qh * 512
                    ps_os = [pso.tile([P, 512], f32, tag="pso", name="pso_t") for _ in range(2)]
                    for kb in range(NKB):
                        if gi + 2 < len(groups):
                            exps_g.setdefault(gi + 2, []).append(st_exp(gi + 2, kb))
                        for par in range(2):
                            nc.tensor.matmul(
                                ps_os[par][0:65, :],
                                lhsT=vext[:, 2 * hp + par, kb, :],
                                rhs=exps_g[gi][kb][:, par * 512 : (par + 1) * 512],
                                start=(kb == 0),
                                stop=(kb == NKB - 1),
                            )
                    for par in range(2):
                        ps_o = ps_os[par]
                        rec = small.tile([1, 512], f32r, tag="rec")
                        with nc.allow_low_precision(reason="denominator reciprocal at tf32 precision"):
                            nc.vector.reciprocal(rec[:], ps_o[64:65, :])
                        rb = recbp.tile([64, 512], f32, tag="recb")
                        nc.tensor.matmul(
                            rb[:], lhsT=ones_col[:], rhs=rec[:], start=True, stop=True
                        )
                        rb_sb = small.tile([64, 512], f32, tag="recb_sb")
                        nc.vector.tensor_copy(rb_sb[:], rb[:])
                        off = par * 64
                        nc.vector.tensor_mul(
                            outT_hp[hp][off : off + 64, qs : qs + 512],
                            ps_o[0:64, :],
                            rb_sb[:],
                        )
                    del exps_g[gi]
                    if hp == 1:
                        # partial proj for this query block (all 4 heads done).
                        # Uses pso-pool psum slots (free right after normalize)
                        # so the 2-slot mm pool stays dedicated to S^T staging.
                        # Staged bf16: halves the output DMA (host sums in f32).
                        for qc in range(4 * qh, 4 * qh + 4):
                            o_sb = ostage.tile([P, C], bf16, tag="osb", name="osb")
                            for cb in range(2):
                                ps = pso.tile([P, 512], f32, tag="pso", name="proj_ps")
                                for kc in range(2):
                                    nc.tensor.matmul(
                                        ps[:],
                                        lhsT=outT_hp[kc][:, qc * P : (qc + 1) * P],
                                        rhs=pj_sb[kc][:, cb * 512 : (cb + 1) * 512],
                                        start=(kc == 0),
                                        stop=(kc == 1),
                                    )
                                nc.vector.tensor_copy(o_sb[:, cb * 512 : (cb + 1) * 512], ps[:])
                            nc.sync.dma_start(out[qc * P : (qc + 1) * P, :], o_sb[:])

    return split_excess_waits(nc)


def shard_inputs(x, spatial_mask, qkv_w, qkv_b, proj_w, g1_w, g1_b, g2_w, g2_b):
    in_maps = []
    for c in range(N_CORES):
        b = c // (N_CORES // B)
        heads = [HPC * (c % (N_CORES // B)) + i for i in range(HPC)]
        dsel = np.array([h * HD + j for h in heads for j in range(HD)])
        sel = np.concatenate([dsel, C + dsel, 2 * C + dsel])
        gconst = np.concatenate(
            [
                qkv_b[sel].reshape(6, P).T,                 # [P, 6]
                spatial_mask[b].reshape(NKB, P).T,          # [P, NKB]
                np.tile(g1_w[:, 0][None, :], (P, 1)),       # [P, GH]
                np.tile(g1_b[None, :], (P, 1)),             # [P, GH]
                np.tile(g2_w[0][None, :], (P, 1)),          # [P, GH]
                np.full((P, 1), g2_b[0], dtype=np.float32),  # [P, 1]
            ],
            axis=1,
        ).astype(np.float32)
        in_maps.append(
            {
                "xbT": np.ascontiguousarray(x[b].T).astype(ml_dtypes.bfloat16),
                "wqkvT": np.ascontiguousarray(qkv_w[sel, :].T).astype(ml_dtypes.bfloat16),
                "projT": np.ascontiguousarray(proj_w[:, dsel].T).astype(ml_dtypes.bfloat16),
                "gconst": np.ascontiguousarray(gconst),
                "ident_in": np.eye(P, dtype=ml_dtypes.bfloat16),
                "ones_in": np.ones((1, 64), dtype=np.float32),
                "vones_in": np.ones((P, HPC, NKB, 1), dtype=ml_dtypes.bfloat16),
            }
        )
    return in_maps


_NC_CACHE = None


def kernel(x, spatial_mask, qkv_w, qkv_b, proj_w, proj_b, g1_w, g1_b, g2_w, g2_b):
    global _NC_CACHE
    x = np.asarray(x, dtype=np.float32)
    spatial_mask = np.asarray(spatial_mask, dtype=np.float32)
    qkv_w = np.asarray(qkv_w, dtype=np.float32)
    qkv_b = np.asarray(qkv_b, dtype=np.float32)
    proj_w = np.asarray(proj_w, dtype=np.float32)
    proj_b = np.asarray(proj_b, dtype=np.float32)
    g1_w = np.asarray(g1_w, dtype=np.float32)
    g1_b = np.asarray(g1_b, dtype=np.float32)
    g2_w = np.asarray(g2_w, dtype=np.float32)
    g2_b = np.asarray(g2_b, dtype=np.float32)

    if _NC_CACHE is None:
        _NC_CACHE = build_nc()
    nc = _NC_CACHE
    in_maps = shard_inputs(
        x, spatial_mask, qkv_w, qkv_b, proj_w, g1_w, g1_b, g2_w, g2_b
    )
    res = run_bass_kernel_spmd(nc, in_maps, list(range(N_CORES)))
    parts = [np.asarray(res.results[c]["out"], dtype=np.float32) for c in range(N_CORES)]
    cpb = N_CORES // B
    full = np.stack(
        [np.sum(parts[b * cpb : (b + 1) * cpb], axis=0) for b in range(B)]
    )
    return (full + proj_b[None, None, :]).astype(np.float32)



# revision 38
# speedup vs baseline: 1.0635x; 1.0635x over previous
"""LocationAwareAttention TRN2 kernel — 8-core SPMD, head+batch sharded.

Sharding: core c handles batch b = c//4 and heads 4*(c%4) .. 4*(c%4)+3.
Each core computes qkv for its 4 heads, gated-softmax attention, and a
partial output projection (its heads' slice of the contraction). The host
sums the 4 partials per batch and adds proj_b.

On-chip layout (per core):
  qkvT  [768, 2048]  rows = [q_h0..q_h3 | k_h0..k_h3 | v_h0..v_h3] x 64 dims
  S^T   [keys, queries] chunks, gate applied as per-partition (key) scale
        inside the Exp activation; softmax denominator via an appended
        ones-column on v (row 64 of the out^T accumulation).
  out^T [65, q] per head accumulated over key blocks; row 64 = denom;
        normalized via DVE reciprocal + a K=1 PE matmul that broadcasts the
        reciprocal row across partitions + DVE mul.
"""

import ml_dtypes
import numpy as np

import concourse.bass as bass
import concourse.mybir as mybir
import concourse.tile as tile
from concourse.bass_utils import run_bass_kernel_spmd
from concourse.vector_clock import ScopedClock

B, N, C = 2, 2048, 1024
H, HD = 16, 64
GH = C // 4
P = 128
HPC = 4          # heads per core
N_CORES = 8
NKB = N // P     # 16 key blocks
NQC = N // P     # 16 query 128-chunks
SCALE = HD ** -0.5

f32 = mybir.dt.float32
f32r = mybir.dt.float32r
bf16 = mybir.dt.bfloat16
AF = mybir.ActivationFunctionType


class SplitDrainTileContext(tile.TileContext):
    """Workaround: this container's walrus rejects >1 sync wait on the Tile
    exit InstDrain ("Too many sync wait commands"). Split the final drain's
    waits across chained single-wait drains."""

    def _drain_and_barrier(self, tick_clock, wait_clock):
        nc = self.nc
        drain_inst = nc.sync.drain()
        wait_clock.add_sem_waits(
            drain_inst.ins, ScopedClock({None: tick_clock.global_clock})
        )
        si = drain_inst.ins.sync_info
        waits = list(si.on_wait) if si and si.on_wait else []
        if len(waits) > 1:
            SyncInfo = type(si)
            drain_inst.ins.sync_info = SyncInfo(
                on_wait=waits[:1], on_update=list(si.on_update)
            )
            for i in range(1, len(waits)):
                extra = nc.sync.drain()
                esi = extra.ins.sync_info
                upd = list(esi.on_update) if esi and esi.on_update else []
                extra.ins.sync_info = SyncInfo(on_wait=waits[i : i + 1], on_update=upd)

        nc.all_engine_barrier()
        assert self.sems is not None
        popped = nc._tile_sem_poison_stack.pop()
        assert popped is self._sem_poison
        nc.clear_and_free_semaphores(list(self.sems.allocated().values()))
        nc.all_engine_barrier()


def split_excess_waits(nc, max_waits=1):
    """This container's walrus rejects instructions carrying more than one
    sync wait. Move excess waits onto same-engine InstNoOps inserted just
    before the instruction (engine streams are in-order, so waiting earlier
    on the same engine is equivalent)."""
    n = 0
    for bb in nc.main_func.blocks:
        il = bb.instructions
        out = []
        changed = False
        for ins in il:
            si = ins.sync_info
            waits = list(si.on_wait) if si and si.on_wait else []
            if len(waits) > max_waits:
                SyncInfo = type(si)
                for i in range(0, len(waits) - max_waits):
                    nop = mybir.InstNoOp(
                        name=f"I-wsplit-{n}",
                        engine=ins.engine,
                        bass_nofuse=True,
                        sync_info=SyncInfo(on_wait=[waits[i]], on_update=[]),
                    )
                    n += 1
                    nc.register_instruction(nop)
                    out.append(nop)
                ins.sync_info = SyncInfo(
                    on_wait=waits[len(waits) - max_waits :],
                    on_update=list(si.on_update),
                )
                changed = True
            out.append(ins)
        if changed:
            bb.instructions = out
    return nc


def build_nc(reps: int = 1):
    nc = bass.Bass()

    xbT = nc.dram_tensor("xbT", [C, N], bf16, kind="ExternalInput")
    wqkvT = nc.dram_tensor("wqkvT", [C, 6 * P], bf16, kind="ExternalInput")
    projT = nc.dram_tensor("projT", [2 * P, C], bf16, kind="ExternalInput")
    # all small f32 constants packed into one tensor -> one startup DMA:
    # cols [qkvb(6) | gmask(16) | g1w(256) | g1b(256) | g2w(256) | g2b(1)]
    GC = 6 + NKB + 3 * GH + 1
    gconst = nc.dram_tensor("gconst", [P, GC], f32, kind="ExternalInput")
    ident_in = nc.dram_tensor("ident_in", [P, P], bf16, kind="ExternalInput")
    ones_in = nc.dram_tensor("ones_in", [1, 64], f32, kind="ExternalInput")
    vones_in = nc.dram_tensor("vones_in", [P, HPC, NKB, 1], bf16, kind="ExternalInput")
    out = nc.dram_tensor("out", [N, C], bf16, kind="ExternalOutput")

    with SplitDrainTileContext(nc) as tc:
        with (
            tc.tile_pool(name="const", bufs=1) as const,
            tc.tile_pool(name="xin", bufs=8) as xin,
            tc.tile_pool(name="qkvt", bufs=1) as qkvt,
            tc.tile_pool(name="vext", bufs=1) as vextp,
            tc.tile_pool(name="exps", bufs=34) as expsp,
            tc.tile_pool(name="outt", bufs=1) as outtp,
            tc.tile_pool(name="small", bufs=4) as small,
            tc.tile_pool(name="ostage", bufs=4) as ostage,
            tc.tile_pool(name="gates", bufs=2) as gatesp,
            tc.tile_pool(name="mm", bufs=2, space="PSUM") as mm,
            tc.tile_pool(name="pso", bufs=3, space="PSUM") as pso,
            tc.tile_pool(name="recb", bufs=1, space="PSUM") as recbp,
        ):
            # ---- constant loads: one packed small-const DMA first so the
            # gate MLP can start while the big weight/x streams stream in ----
            gc_sb = const.tile([P, GC], f32, tag="gc")
            nc.sync.dma_start(gc_sb[:], gconst[:])
            qkvb_sb = [gc_sb[:, m : m + 1] for m in range(6)]
            gm_sb = gc_sb[:, 6 : 6 + NKB]
            o1 = 6 + NKB
            g1w_sb = gc_sb[:, o1 : o1 + GH]
            g1b_sb = gc_sb[:, o1 + GH : o1 + 2 * GH]
            g2w_sb = gc_sb[:, o1 + 2 * GH : o1 + 3 * GH]
            g2b_sb = gc_sb[:, o1 + 3 * GH : o1 + 3 * GH + 1]
            ident = const.tile([P, P], bf16, tag="ident")
            nc.sync.dma_start(ident[:], ident_in[:])
            ones_col = const.tile([1, 64], f32r, tag="ones_col")
            nc.sync.dma_start(ones_col[:], ones_in[:].bitcast(f32r))
            w_sb = []
            for kc in range(C // P):
                t = const.tile([P, 6 * P], bf16, tag=f"w{kc}", name=f"w{kc}")
                nc.sync.dma_start(t[:], wqkvT[kc * P : (kc + 1) * P, :])
                w_sb.append(t)
            pj_sb = []
            for kc in range(2):
                t = const.tile([P, C], bf16, tag=f"pj{kc}", name=f"pj{kc}")
                pj_sb.append(t)

            for rep in range(reps):
                x_sb = []
                for kc in range(C // P):
                    t = xin.tile([P, N], bf16, tag="xt")
                    nc.sync.dma_start(t[:], xbT[kc * P : (kc + 1) * P, :])
                    x_sb.append(t)
                if rep == 0:
                    # projT queued after x: it is not needed until the first
                    # proj (~150us in) and would delay the startup x stream
                    for kc in range(2):
                        nc.sync.dma_start(pj_sb[kc][:], projT[kc * P : (kc + 1) * P, :])

                # ---- spatial gate: gatesc[k] = 0.125 * sigmoid(mlp(mask[k])) ----
                # double-buffered so rep r+1's gate can run while rep r's
                # exps still read the previous gatesc
                gatesc = gatesp.tile([P, NKB], f32, tag="gatesc")
                for kb in range(NKB):
                    m_col = gc_sb[:, 6 + kb : 7 + kb]
                    t1 = gatesp.tile([P, GH], f32, tag="g_t1")
                    nc.vector.tensor_scalar_mul(t1[:], g1w_sb, m_col)
                    nc.vector.tensor_add(t1[:], t1[:], g1b_sb)
                    nc.scalar.activation(t1[:], t1[:], AF.Relu)
                    nc.vector.tensor_mul(t1[:], t1[:], g2w_sb)
                    gp = gatesp.tile([P, 1], f32, tag="g_gp")
                    nc.vector.reduce_sum(gp[:], t1[:], axis=mybir.AxisListType.X)
                    nc.scalar.activation(gp[:], gp[:], AF.Sigmoid, bias=g2b_sb)
                    nc.scalar.mul(gatesc[:, kb : kb + 1], gp[:], SCALE)

                # ---- qkvT = W_sel @ x_b^T  (+bias) : [768, 2048] ----
                # Emitted in (m, nf, half) units of 8 matmuls so S^T+exp
                # work can interleave between units at ~1.7us granularity.
                qkvT_m = {m: qkvt.tile([P, N], bf16, tag=f"qkvT{m}", name=f"qkvT{m}") for m in range(6)}

                def emit_qkv_unit(m, q):
                    # one self-contained 512-column block: 8 accumulating
                    # matmuls into a 1-bank pso tile + fused bias-add drain
                    ps = pso.tile([P, 512], f32, tag="pso", name="qkv_ps")
                    for kc in range(C // P):
                        nc.tensor.matmul(
                            ps[:],
                            lhsT=w_sb[kc][:, m * P : (m + 1) * P],
                            rhs=x_sb[kc][:, q * 512 : (q + 1) * 512],
                            start=(kc == 0),
                            stop=(kc == C // P - 1),
                        )
                    nc.vector.tensor_scalar_add(
                        qkvT_m[m][:, q * 512 : (q + 1) * 512], ps[:], qkvb_sb[m]
                    )

                def emit_qkv(m):
                    for q in range(4):
                        emit_qkv_unit(m, q)

                # ---- attention: software-pipelined over 8 (qh, hp) groups ----
                # Two groups of S^T + gated exp are prologued between qkv
                # emissions so ACT always has a ~16-tile backlog; in the main
                # loop stage A of group g+2 is emitted interleaved per key
                # block with stage B of group g (out^T accumulation).
                outT_hp = [outtp.tile([P, N], bf16, tag=f"outT{i}", name=f"outT{i}") for i in range(2)]
                groups = [(qh, hp) for qh in range(4) for hp in range(2)]

                def st_exp(gi, kb):
                    qh, hp = groups[gi]
                    qm, km = hp, 2 + hp
                    qs = qh * 512
                    ps = mm.tile([P, 1024], f32, tag="mmt", name="st_ps")
                    for par in range(2):
                        nc.tensor.matmul(
                            ps[:, par * 512 : (par + 1) * 512],
                            lhsT=qkvT_m[km][par * 64 : par * 64 + 64, kb * P : (kb + 1) * P],
                            rhs=qkvT_m[qm][par * 64 : par * 64 + 64, qs : qs + 512],
                            start=True,
                            stop=True,
                        )
                    e = expsp.tile([P, 1024], bf16, tag="exps", name="exps_t")
                    nc.scalar.activation(e[:], ps[:], AF.Exp, scale=gatesc[:, kb : kb + 1])
                    return e

                # q/k for head pair 0 first (DMA-paced), then the remaining
                # qkv units interleaved 2 S^T+exp pairs per 8-matmul unit so
                # PE never idles while ACT digests the exp backlog (the
                # 2-slot mm pool caps ACT lookahead, so st emission would
                # otherwise pace PE at exp speed).
                # boot: qkv for m=0,2 processed kc-major across 8 one-bank
                # psum slots (3 pso + 2x2 mm halves + 1 recb) so PE computes
                # at x-stream pace instead of stalling for the full x load
                boot_ps = {}
                slots = []
                for _ in range(3):
                    bp = pso.tile([P, 512], f32, tag="pso", name="boot_ps")
                    slots.append(bp)
                for _ in range(2):
                    bm = mm.tile([P, 1024], f32, tag="mmt", name="boot_mm")
                    slots.append(bm[:, 0:512])
                    slots.append(bm[:, 512:1024])
                br = recbp.tile([P, 512], f32, tag="recb", name="boot_rec")
                slots.append(br)
                boot_mq = [(m, q) for m in (0, 2) for q in range(4)]
                for si, (m, q) in enumerate(boot_mq):
                    boot_ps[m, q] = slots[si]
                for kc in range(C // P):
                    for m, q in boot_mq:
                        nc.tensor.matmul(
                            boot_ps[m, q][:],
                            lhsT=w_sb[kc][:, m * P : (m + 1) * P],
                            rhs=x_sb[kc][:, q * 512 : (q + 1) * 512],
                            start=(kc == 0),
                            stop=(kc == C // P - 1),
                        )
                for m, q in boot_mq:
                    nc.vector.tensor_scalar_add(
                        qkvT_m[m][:, q * 512 : (q + 1) * 512], boot_ps[m, q][:], qkvb_sb[m]
                    )

                exps_g = {0: [], 1: []}
                units = [(m, q) for m in (1, 3, 4, 5) for q in range(4)]
                sts = [(0, kb) for kb in range(NKB)] + [(1, kb) for kb in range(NKB)]
                for ui, (m, q) in enumerate(units):
                    emit_qkv_unit(m, q)
                    for gi, kb in sts[2 * ui : 2 * ui + 2]:
                        exps_g[gi].append(st_exp(gi, kb))

                # ---- v^T -> v transpose, build vext [keys, 65] per head (bf16) ----
                vext = vextp.tile([P, HPC, NKB, 65], bf16, tag="vext")
                nc.sync.dma_start(vext[:, :, :, 64:65], vones_in[:])
                with nc.allow_low_precision(reason="pure transpose, no accumulation"):
                    for vc in range(2):  # qkvT chunks 4,5 hold [v_h0|v_h1], [v_h2|v_h3]
                        for g in range(2):  # groups of 8 key blocks share one psum tile
                            ps = mm.tile([P, 1024], bf16, tag="mmt", name="tr_ps")
                            for kk in range(8):
                                kb = g * 8 + kk
                                nc.tensor.transpose(
                                    ps[:, kk * P : (kk + 1) * P],
                                    qkvT_m[4 + vc][:, kb * P : (kb + 1) * P],
                                    ident[:],
                                )
                            for kk in range(8):
                                kb = g * 8 + kk
                                for half in range(2):
                                    nc.vector.tensor_copy(
                                        vext[:, 2 * vc + half, kb, 0:64],
                                        ps[:, kk * P + half * 64 : kk * P + half * 64 + 64],
                                    )

                for gi in range(len(groups)):
                    qh, hp = groups[gi]
                    qs = qh * 512
                    ps_os = [pso.tile([P, 512], f32, tag="pso", name="pso_t") for _ in range(2)]
                    for kb in range(NKB):
                        if gi + 2 < len(groups):
                            exps_g.setdefault(gi + 2, []).append(st_exp(gi + 2, kb))
                        for par in range(2):
                            nc.tensor.matmul(
                                ps_os[par][0:65, :],
                                lhsT=vext[:, 2 * hp + par, kb, :],
                                rhs=exps_g[gi][kb][:, par * 512 : (par + 1) * 512],
                                start=(kb == 0),
                                stop=(kb == NKB - 1),
                            )
                    for par in range(2):
                        ps_o = ps_os[par]
                        rec = small.tile([1, 512], f32r, tag="rec")
                        with nc.allow_low_precision(reason="denominator reciprocal at tf32 precision"):
                            nc.vector.reciprocal(rec[:], ps_o[64:65, :])
                        rb = recbp.tile([64, 512], f32, tag="recb")
                        nc.tensor.matmul(
                            rb[:], lhsT=ones_col[:], rhs=rec[:], start=True, stop=True
                        )
                        rb_sb = small.tile([64, 512], f32, tag="recb_sb")
                        nc.vector.tensor_copy(rb_sb[:], rb[:])
                        off = par * 64
                        nc.vector.tensor_mul(
                            outT_hp[hp][off : off + 64, qs : qs + 512],
                            ps_o[0:64, :],
                            rb_sb[:],
                        )
                    del exps_g[gi]
                    if hp == 1:
                        # partial proj for this query block (all 4 heads done).
                        # Uses pso-pool psum slots (free right after normalize)
                        # so the 2-slot mm pool stays dedicated to S^T staging.
                        # Staged bf16: halves the output DMA (host sums in f32).
                        for qc in range(4 * qh, 4 * qh + 4):
                            o_sb = ostage.tile([P, C], bf16, tag="osb", name="osb")
                            for cb in range(2):
                                ps = pso.tile([P, 512], f32, tag="pso", name="proj_ps")
                                for kc in range(2):
                                    nc.tensor.matmul(
                                        ps[:],
                                        lhsT=outT_hp[kc][:, qc * P : (qc + 1) * P],
                                        rhs=pj_sb[kc][:, cb * 512 : (cb + 1) * 512],
                                        start=(kc == 0),
                                        stop=(kc == 1),
                                    )
                                nc.vector.tensor_copy(o_sb[:, cb * 512 : (cb + 1) * 512], ps[:])
                            nc.sync.dma_start(out[qc * P : (qc + 1) * P, :], o_sb[:])

    return split_excess_waits(nc)


def shard_inputs(x, spatial_mask, qkv_w, qkv_b, proj_w, g1_w, g1_b, g2_w, g2_b):
    in_maps = []
    for c in range(N_CORES):
        b = c // (N_CORES // B)
        heads = [HPC * (c % (N_CORES // B)) + i for i in range(HPC)]
        dsel = np.array([h * HD + j for h in heads for j in range(HD)])
        sel = np.concatenate([dsel, C + dsel, 2 * C + dsel])
        gconst = np.concatenate(
            [
                qkv_b[sel].reshape(6, P).T,                 # [P, 6]
                spatial_mask[b].reshape(NKB, P).T,          # [P, NKB]
                np.tile(g1_w[:, 0][None, :], (P, 1)),       # [P, GH]
                np.tile(g1_b[None, :], (P, 1)),             # [P, GH]
                np.tile(g2_w[0][None, :], (P, 1)),          # [P, GH]
                np.full((P, 1), g2_b[0], dtype=np.float32),  # [P, 1]
            ],
            axis=1,
        ).astype(np.float32)
        in_maps.append(
            {
                "xbT": np.ascontiguousarray(x[b].T).astype(ml_dtypes.bfloat16),
                "wqkvT": np.ascontiguousarray(qkv_w[sel, :].T).astype(ml_dtypes.bfloat16),
                "projT": np.ascontiguousarray(proj_w[:, dsel].T).astype(ml_dtypes.bfloat16),
                "gconst": np.ascontiguousarray(gconst),
                "ident_in": np.eye(P, dtype=ml_dtypes.bfloat16),
                "ones_in": np.ones((1, 64), dtype=np.float32),
                "vones_in": np.ones((P, HPC, NKB, 1), dtype=ml_dtypes.bfloat16),
            }
        )
    return in_maps


_NC_CACHE = None


def kernel(x, spatial_mask, qkv_w, qkv_b, proj_w, proj_b, g1_w, g1_b, g2_w, g2_b):
    global _NC_CACHE
    x = np.asarray(x, dtype=np.float32)
    spatial_mask = np.asarray(spatial_mask, dtype=np.float32)
    qkv_w = np.asarray(qkv_w, dtype=np.float32)
    qkv_b = np.asarray(qkv_b, dtype=np.float32)
    proj_w = np.asarray(proj_w, dtype=np.float32)
    proj_b = np.asarray(proj_b, dtype=np.float32)
    g1_w = np.asarray(g1_w, dtype=np.float32)
    g1_b = np.asarray(g1_b, dtype=np.float32)
    g2_w = np.asarray(g2_w, dtype=np.float32)
    g2_b = np.asarray(g2_b, dtype=np.float32)

    if _NC_CACHE is None:
        _NC_CACHE = build_nc()
    nc = _NC_CACHE
    in_maps = shard_inputs(
        x, spatial_mask, qkv_w, qkv_b, proj_w, g1_w, g1_b, g2_w, g2_b
    )
    res = run_bass_kernel_spmd(nc, in_maps, list(range(N_CORES)))
    parts = [np.asarray(res.results[c]["out"], dtype=np.float32) for c in range(N_CORES)]
    cpb = N_CORES // B
    full = np.stack(
        [np.sum(parts[b * cpb : (b + 1) * cpb], axis=0) for b in range(B)]
    )
    return (full + proj_b[None, None, :]).astype(np.float32)



# revision 42
# speedup vs baseline: 1.2477x; 1.1733x over previous
"""LocationAwareAttention TRN2 kernel — 8-core SPMD, head+batch sharded.

Sharding: core c handles batch b = c//4 and heads 4*(c%4) .. 4*(c%4)+3.
Each core computes qkv for its 4 heads, gated-softmax attention, and a
partial output projection (its heads' slice of the contraction). The host
sums the 4 partials per batch and adds proj_b.

On-chip layout (per core):
  qkvT  [768, 2048]  rows = [q_h0..q_h3 | k_h0..k_h3 | v_h0..v_h3] x 64 dims
  S^T   [keys, queries] chunks, gate applied as per-partition (key) scale
        inside the Exp activation; softmax denominator via an appended
        ones-column on v (row 64 of the out^T accumulation).
  out^T [65, q] per head accumulated over key blocks; row 64 = denom;
        normalized via DVE reciprocal + a K=1 PE matmul that broadcasts the
        reciprocal row across partitions + DVE mul.
"""

import ml_dtypes
import numpy as np

import concourse.bass as bass
import concourse.mybir as mybir
import concourse.tile as tile
from concourse.bass_utils import run_bass_kernel_spmd
from concourse.vector_clock import ScopedClock

B, N, C = 2, 2048, 1024
H, HD = 16, 64
GH = C // 4
P = 128
HPC = 4          # heads per core
N_CORES = 8
NKB = N // P     # 16 key blocks
NQC = N // P     # 16 query 128-chunks
SCALE = HD ** -0.5

f32 = mybir.dt.float32
f32r = mybir.dt.float32r
bf16 = mybir.dt.bfloat16
AF = mybir.ActivationFunctionType


class SplitDrainTileContext(tile.TileContext):
    """Workaround: this container's walrus rejects >1 sync wait on the Tile
    exit InstDrain ("Too many sync wait commands"). Split the final drain's
    waits across chained single-wait drains."""

    def _drain_and_barrier(self, tick_clock, wait_clock):
        nc = self.nc
        drain_inst = nc.sync.drain()
        wait_clock.add_sem_waits(
            drain_inst.ins, ScopedClock({None: tick_clock.global_clock})
        )
        si = drain_inst.ins.sync_info
        waits = list(si.on_wait) if si and si.on_wait else []
        if len(waits) > 1:
            SyncInfo = type(si)
            drain_inst.ins.sync_info = SyncInfo(
                on_wait=waits[:1], on_update=list(si.on_update)
            )
            for i in range(1, len(waits)):
                extra = nc.sync.drain()
                esi = extra.ins.sync_info
                upd = list(esi.on_update) if esi and esi.on_update else []
                extra.ins.sync_info = SyncInfo(on_wait=waits[i : i + 1], on_update=upd)

        nc.all_engine_barrier()
        assert self.sems is not None
        popped = nc._tile_sem_poison_stack.pop()
        assert popped is self._sem_poison
        nc.clear_and_free_semaphores(list(self.sems.allocated().values()))
        nc.all_engine_barrier()


def split_excess_waits(nc, max_waits=1):
    """This container's walrus rejects instructions carrying more than one
    sync wait. Move excess waits onto same-engine InstNoOps inserted just
    before the instruction (engine streams are in-order, so waiting earlier
    on the same engine is equivalent)."""
    n = 0
    for bb in nc.main_func.blocks:
        il = bb.instructions
        out = []
        changed = False
        for ins in il:
            si = ins.sync_info
            waits = list(si.on_wait) if si and si.on_wait else []
            if len(waits) > max_waits:
                SyncInfo = type(si)
                for i in range(0, len(waits) - max_waits):
                    nop = mybir.InstNoOp(
                        name=f"I-wsplit-{n}",
                        engine=ins.engine,
                        bass_nofuse=True,
                        sync_info=SyncInfo(on_wait=[waits[i]], on_update=[]),
                    )
                    n += 1
                    nc.register_instruction(nop)
                    out.append(nop)
                ins.sync_info = SyncInfo(
                    on_wait=waits[len(waits) - max_waits :],
                    on_update=list(si.on_update),
                )
                changed = True
            out.append(ins)
        if changed:
            bb.instructions = out
    return nc


def build_nc(reps: int = 1):
    nc = bass.Bass()

    xbT = nc.dram_tensor("xbT", [C, N], bf16, kind="ExternalInput")
    wqkvT = nc.dram_tensor("wqkvT", [C, 6 * P], bf16, kind="ExternalInput")
    projT = nc.dram_tensor("projT", [2 * P, C], bf16, kind="ExternalInput")
    # all small f32 constants packed into one tensor -> one startup DMA:
    # cols [qkvb(6) | gmask(16) | g1w(256) | g1b(256) | g2w(256) | g2b(1)]
    GC = 6 + NKB + 3 * GH + 1
    gconst = nc.dram_tensor("gconst", [P, GC], f32, kind="ExternalInput")
    ident_in = nc.dram_tensor("ident_in", [P, P], bf16, kind="ExternalInput")
    ones_in = nc.dram_tensor("ones_in", [1, 64], f32, kind="ExternalInput")
    vones_in = nc.dram_tensor("vones_in", [P, HPC, NKB, 1], bf16, kind="ExternalInput")
    out = nc.dram_tensor("out", [N, C], bf16, kind="ExternalOutput")

    with SplitDrainTileContext(nc) as tc:
        with (
            tc.tile_pool(name="const", bufs=1) as const,
            tc.tile_pool(name="xin", bufs=8) as xin,
            tc.tile_pool(name="qkvt", bufs=1) as qkvt,
            tc.tile_pool(name="vext", bufs=1) as vextp,
            tc.tile_pool(name="exps", bufs=34) as expsp,
            tc.tile_pool(name="outt", bufs=1) as outtp,
            tc.tile_pool(name="small", bufs=4) as small,
            tc.tile_pool(name="ostage", bufs=4) as ostage,
            tc.tile_pool(name="gates", bufs=2) as gatesp,
            tc.tile_pool(name="mm", bufs=2, space="PSUM") as mm,
            tc.tile_pool(name="pso", bufs=3, space="PSUM") as pso,
            tc.tile_pool(name="recb", bufs=1, space="PSUM") as recbp,
        ):
            # ---- constant loads: one packed small-const DMA first so the
            # gate MLP can start while the big weight/x streams stream in ----
            gc_sb = const.tile([P, GC], f32, tag="gc")
            nc.sync.dma_start(gc_sb[:], gconst[:])
            qkvb_sb = [gc_sb[:, m : m + 1] for m in range(6)]
            gm_sb = gc_sb[:, 6 : 6 + NKB]
            o1 = 6 + NKB
            g1w_sb = gc_sb[:, o1 : o1 + GH]
            g1b_sb = gc_sb[:, o1 + GH : o1 + 2 * GH]
            g2w_sb = gc_sb[:, o1 + 2 * GH : o1 + 3 * GH]
            g2b_sb = gc_sb[:, o1 + 3 * GH : o1 + 3 * GH + 1]
            ident = const.tile([P, P], bf16, tag="ident")
            nc.sync.dma_start(ident[:], ident_in[:])
            ones_col = const.tile([1, 64], f32r, tag="ones_col")
            nc.sync.dma_start(ones_col[:], ones_in[:].bitcast(f32r))
            w_sb = []
            for kc in range(C // P):
                t = const.tile([P, 6 * P], bf16, tag=f"w{kc}", name=f"w{kc}")
                nc.sync.dma_start(t[:], wqkvT[kc * P : (kc + 1) * P, :])
                w_sb.append(t)
            pj_sb = []
            for kc in range(2):
                t = const.tile([P, C], bf16, tag=f"pj{kc}", name=f"pj{kc}")
                pj_sb.append(t)

            for rep in range(reps):
                x_sb = []
                for kc in range(C // P):
                    t = xin.tile([P, N], bf16, tag="xt")
                    nc.sync.dma_start(t[:], xbT[kc * P : (kc + 1) * P, :])
                    x_sb.append(t)
                if rep == 0:
                    # projT queued after x: it is not needed until the first
                    # proj (~150us in) and would delay the startup x stream
                    for kc in range(2):
                        nc.sync.dma_start(pj_sb[kc][:], projT[kc * P : (kc + 1) * P, :])

                # ---- spatial gate: gatesc[k] = 0.125 * sigmoid(mlp(mask[k])) ----
                # double-buffered so rep r+1's gate can run while rep r's
                # exps still read the previous gatesc
                gatesc = gatesp.tile([P, NKB], f32, tag="gatesc")
                for kb in range(NKB):
                    m_col = gc_sb[:, 6 + kb : 7 + kb]
                    t1 = gatesp.tile([P, GH], f32, tag="g_t1")
                    nc.vector.tensor_scalar_mul(t1[:], g1w_sb, m_col)
                    nc.vector.tensor_add(t1[:], t1[:], g1b_sb)
                    nc.scalar.activation(t1[:], t1[:], AF.Relu)
                    nc.vector.tensor_mul(t1[:], t1[:], g2w_sb)
                    gp = gatesp.tile([P, 1], f32, tag="g_gp")
                    nc.vector.reduce_sum(gp[:], t1[:], axis=mybir.AxisListType.X)
                    nc.scalar.activation(gp[:], gp[:], AF.Sigmoid, bias=g2b_sb)
                    nc.scalar.mul(gatesc[:, kb : kb + 1], gp[:], SCALE)

                # ---- qkvT = W_sel @ x_b^T  (+bias) : [768, 2048] ----
                # Emitted in (m, nf, half) units of 8 matmuls so S^T+exp
                # work can interleave between units at ~1.7us granularity.
                qkvT_m = {m: qkvt.tile([P, N], bf16, tag=f"qkvT{m}", name=f"qkvT{m}") for m in range(6)}

                def emit_qkv_unit(m, q):
                    # one self-contained 512-column block: 8 accumulating
                    # matmuls into a 1-bank pso tile + fused bias-add drain
                    ps = pso.tile([P, 512], f32, tag="pso", name="qkv_ps")
                    for kc in range(C // P):
                        nc.tensor.matmul(
                            ps[:],
                            lhsT=w_sb[kc][:, m * P : (m + 1) * P],
                            rhs=x_sb[kc][:, q * 512 : (q + 1) * 512],
                            start=(kc == 0),
                            stop=(kc == C // P - 1),
                        )
                    nc.vector.tensor_scalar_add(
                        qkvT_m[m][:, q * 512 : (q + 1) * 512], ps[:], qkvb_sb[m]
                    )

                def emit_qkv(m):
                    for q in range(4):
                        emit_qkv_unit(m, q)

                # ---- attention: software-pipelined over 8 (qh, hp) groups ----
                # Two groups of S^T + gated exp are prologued between qkv
                # emissions so ACT always has a ~16-tile backlog; in the main
                # loop stage A of group g+2 is emitted interleaved per key
                # block with stage B of group g (out^T accumulation).
                outT_hp = [outtp.tile([P, N], bf16, tag=f"outT{i}", name=f"outT{i}") for i in range(2)]
                groups = [(qh, hp) for qh in range(4) for hp in range(2)]

                def st_exp(gi, kb):
                    qh, hp = groups[gi]
                    qm, km = hp, 2 + hp
                    qs = qh * 512
                    ps = mm.tile([P, 1024], f32, tag="mmt", name="st_ps")
                    for par in range(2):
                        nc.tensor.matmul(
                            ps[:, par * 512 : (par + 1) * 512],
                            lhsT=qkvT_m[km][par * 64 : par * 64 + 64, kb * P : (kb + 1) * P],
                            rhs=qkvT_m[qm][par * 64 : par * 64 + 64, qs : qs + 512],
                            start=True,
                            stop=True,
                        )
                    e = expsp.tile([P, 1024], bf16, tag="exps", name="exps_t")
                    nc.scalar.activation(e[:], ps[:], AF.Exp, scale=gatesc[:, kb : kb + 1])
                    return e

                # q/k for head pair 0 first (DMA-paced), then the remaining
                # qkv units interleaved 2 S^T+exp pairs per 8-matmul unit so
                # PE never idles while ACT digests the exp backlog (the
                # 2-slot mm pool caps ACT lookahead, so st emission would
                # otherwise pace PE at exp speed).
                # boot: qkv for m=0,2 processed kc-major across 8 one-bank
                # psum slots (3 pso + 2x2 mm halves + 1 recb) so PE computes
                # at x-stream pace instead of stalling for the full x load
                boot_ps = {}
                slots = []
                for _ in range(3):
                    bp = pso.tile([P, 512], f32, tag="pso", name="boot_ps")
                    slots.append(bp)
                for _ in range(2):
                    bm = mm.tile([P, 1024], f32, tag="mmt", name="boot_mm")
                    slots.append(bm[:, 0:512])
                    slots.append(bm[:, 512:1024])
                br = recbp.tile([P, 512], f32, tag="recb", name="boot_rec")
                slots.append(br)
                boot_mq = [(m, q) for m in (0, 2) for q in range(4)]
                for si, (m, q) in enumerate(boot_mq):
                    boot_ps[m, q] = slots[si]
                for kc in range(C // P):
                    for m, q in boot_mq:
                        nc.tensor.matmul(
                            boot_ps[m, q][:],
                            lhsT=w_sb[kc][:, m * P : (m + 1) * P],
                            rhs=x_sb[kc][:, q * 512 : (q + 1) * 512],
                            start=(kc == 0),
                            stop=(kc == C // P - 1),
                        )
                for m, q in boot_mq:
                    nc.vector.tensor_scalar_add(
                        qkvT_m[m][:, q * 512 : (q + 1) * 512], boot_ps[m, q][:], qkvb_sb[m]
                    )

                exps_g = {0: [], 1: []}
                units = [(m, q) for m in (1, 3, 4, 5) for q in range(4)]
                sts = [(0, kb) for kb in range(NKB)] + [(1, kb) for kb in range(NKB)]
                for ui, (m, q) in enumerate(units):
                    emit_qkv_unit(m, q)
                    for gi, kb in sts[2 * ui : 2 * ui + 2]:
                        exps_g[gi].append(st_exp(gi, kb))

                # ---- v^T -> v transpose, build vext [keys, 65] per head (bf16) ----
                vext = vextp.tile([P, HPC, NKB, 65], bf16, tag="vext")
                nc.sync.dma_start(vext[:, :, :, 64:65], vones_in[:])
                with nc.allow_low_precision(reason="pure transpose, no accumulation"):
                    for vc in range(2):  # qkvT chunks 4,5 hold [v_h0|v_h1], [v_h2|v_h3]
                        for g in range(2):  # groups of 8 key blocks share one psum tile
                            ps = mm.tile([P, 1024], bf16, tag="mmt", name="tr_ps")
                            for kk in range(8):
                                kb = g * 8 + kk
                                nc.tensor.transpose(
                                    ps[:, kk * P : (kk + 1) * P],
                                    qkvT_m[4 + vc][:, kb * P : (kb + 1) * P],
                                    ident[:],
                                )
                            for kk in range(8):
                                kb = g * 8 + kk
                                for half in range(2):
                                    nc.vector.tensor_copy(
                                        vext[:, 2 * vc + half, kb, 0:64],
                                        ps[:, kk * P + half * 64 : kk * P + half * 64 + 64],
                                    )

                for gi in range(len(groups)):
                    qh, hp = groups[gi]
                    qs = qh * 512
                    ps_os = [pso.tile([P, 512], f32, tag="pso", name="pso_t") for _ in range(2)]
                    for kb in range(NKB):
                        if gi + 2 < len(groups):
                            exps_g.setdefault(gi + 2, []).append(st_exp(gi + 2, kb))
                        for par in range(2):
                            nc.tensor.matmul(
                                ps_os[par][0:65, :],
                                lhsT=vext[:, 2 * hp + par, kb, :],
                                rhs=exps_g[gi][kb][:, par * 512 : (par + 1) * 512],
                                start=(kb == 0),
                                stop=(kb == NKB - 1),
                            )
                    for par in range(2):
                        ps_o = ps_os[par]
                        rec = small.tile([1, 512], f32r, tag="rec")
                        with nc.allow_low_precision(reason="denominator reciprocal at tf32 precision"):
                            nc.vector.reciprocal(rec[:], ps_o[64:65, :])
                        rb = recbp.tile([64, 512], f32, tag="recb")
                        nc.tensor.matmul(
                            rb[:], lhsT=ones_col[:], rhs=rec[:], start=True, stop=True
                        )
                        rb_sb = small.tile([64, 512], f32, tag="recb_sb")
                        nc.vector.tensor_copy(rb_sb[:], rb[:])
                        off = par * 64
                        nc.vector.tensor_mul(
                            outT_hp[hp][off : off + 64, qs : qs + 512],
                            ps_o[0:64, :],
                            rb_sb[:],
                        )
                    del exps_g[gi]
                    if hp == 1:
                        # partial proj for this query block (all 4 heads done).
                        # Uses pso-pool psum slots (free right after normalize)
                        # so the 2-slot mm pool stays dedicated to S^T staging.
                        # Staged bf16: halves the output DMA (host sums in f32).
                        for qc in range(4 * qh, 4 * qh + 4):
                            o_sb = ostage.tile([P, C], bf16, tag="osb", name="osb")
                            for cb in range(2):
                                ps = pso.tile([P, 512], f32, tag="pso", name="proj_ps")
                                for kc in range(2):
                                    nc.tensor.matmul(
                                        ps[:],
                                        lhsT=outT_hp[kc][:, qc * P : (qc + 1) * P],
                                        rhs=pj_sb[kc][:, cb * 512 : (cb + 1) * 512],
                                        start=(kc == 0),
                                        stop=(kc == 1),
                                    )
                                nc.vector.tensor_copy(o_sb[:, cb * 512 : (cb + 1) * 512], ps[:])
                            nc.sync.dma_start(out[qc * P : (qc + 1) * P, :], o_sb[:])

    return split_excess_waits(nc)


def shard_inputs(x, spatial_mask, qkv_w, qkv_b, proj_w, g1_w, g1_b, g2_w, g2_b):
    in_maps = []
    for c in range(N_CORES):
        b = c // (N_CORES // B)
        heads = [HPC * (c % (N_CORES // B)) + i for i in range(HPC)]
        dsel = np.array([h * HD + j for h in heads for j in range(HD)])
        sel = np.concatenate([dsel, C + dsel, 2 * C + dsel])
        gconst = np.concatenate(
            [
                qkv_b[sel].reshape(6, P).T,                 # [P, 6]
                spatial_mask[b].reshape(NKB, P).T,          # [P, NKB]
                np.tile(g1_w[:, 0][None, :], (P, 1)),       # [P, GH]
                np.tile(g1_b[None, :], (P, 1)),             # [P, GH]
                np.tile(g2_w[0][None, :], (P, 1)),          # [P, GH]
                np.full((P, 1), g2_b[0], dtype=np.float32),  # [P, 1]
            ],
            axis=1,
        ).astype(np.float32)
        in_maps.append(
            {
                "xbT": np.ascontiguousarray(x[b].T).astype(ml_dtypes.bfloat16),
                "wqkvT": np.ascontiguousarray(qkv_w[sel, :].T).astype(ml_dtypes.bfloat16),
                "projT": np.ascontiguousarray(proj_w[:, dsel].T).astype(ml_dtypes.bfloat16),
                "gconst": np.ascontiguousarray(gconst),
                "ident_in": np.eye(P, dtype=ml_dtypes.bfloat16),
                "ones_in": np.ones((1, 64), dtype=np.float32),
                "vones_in": np.ones((P, HPC, NKB, 1), dtype=ml_dtypes.bfloat16),
            }
        )
    return in_maps


_NC_CACHE = None


def kernel(x, spatial_mask, qkv_w, qkv_b, proj_w, proj_b, g1_w, g1_b, g2_w, g2_b):
    global _NC_CACHE
    x = np.asarray(x, dtype=np.float32)
    spatial_mask = np.asarray(spatial_mask, dtype=np.float32)
    qkv_w = np.asarray(qkv_w, dtype=np.float32)
    qkv_b = np.asarray(qkv_b, dtype=np.float32)
    proj_w = np.asarray(proj_w, dtype=np.float32)
    proj_b = np.asarray(proj_b, dtype=np.float32)
    g1_w = np.asarray(g1_w, dtype=np.float32)
    g1_b = np.asarray(g1_b, dtype=np.float32)
    g2_w = np.asarray(g2_w, dtype=np.float32)
    g2_b = np.asarray(g2_b, dtype=np.float32)

    if _NC_CACHE is None:
        _NC_CACHE = build_nc()
    nc = _NC_CACHE
    in_maps = shard_inputs(
        x, spatial_mask, qkv_w, qkv_b, proj_w, g1_w, g1_b, g2_w, g2_b
    )
    res = run_bass_kernel_spmd(nc, in_maps, list(range(N_CORES)))
    parts = [np.asarray(res.results[c]["out"], dtype=np.float32) for c in range(N_CORES)]
    cpb = N_CORES // B
    full = np.stack(
        [np.sum(parts[b * cpb : (b + 1) * cpb], axis=0) for b in range(B)]
    )
    return (full + proj_b[None, None, :]).astype(np.float32)

